# revision 8
# baseline (speedup 1.0000x reference)
"""Trainium2 Bass kernel for nn_Cropper: 100 bilinear 100x100 crops per image,
8 images data-parallel across 8 NeuronCores.

Per core, per box (c-interleaved layout):
  - Host precomputes all indices/weights; image shipped bf16 as [W, H, C]
    (x-major; one "column" = H*C contiguous (y, c) pairs).
  - dma_gather (GB boxes/instr): descriptor per (box, u, j) fetches the x-tap
    column window: 1536 bf16 elements starting 128-aligned inside the column,
    covering the <=462-row y-span times 3 channels. Partition = j.
  - Horizontal blend on DVE bf16 with per-partition scalars (wx per j):
    h = G_u0*(1-wx) + G_u1*wx.
  - ACT upcast h -> f32 (ap_gather moves 4-byte units; d=3 f32 = one (y, *)
    channel triple).
  - ap_gather (AGB boxes/instr): 200 idx/box pulls (t, i) vertical taps as
    d=3 channel triples from the f32 table.
  - Final vertical blend on DVE f32: o = Hv0 + (Hv1-Hv0)*wy  -> [j, i, c].
  - PE transposes [j, i] -> [i, j] per channel; ACT copies PSUM->SBUF;
    DMA writes [i, c, j] to out[m, c, i, j].
"""
import numpy as np
import ml_dtypes
from contextlib import ExitStack

B, NBOX, C, H, W = 8, 100, 3, 1024, 1024
S = 100
ELEM = 1920         # dma_gather element size (bf16): covers (462+1+127)*3, mult of 384
STEP = 128          # virtual row granularity (256B bf16)
YW = ELEM // C      # 512 y rows in the gathered window
NIDX = 2 * 128      # dma_gather descriptors per box (u, j)
NAG = 200           # ap_gather indices per box (t, i)
NPAD = 2048         # pad elements at end of image buffer
GB = 2              # boxes per dma_gather instruction
AGB = 4             # boxes per ap_gather instruction
GIW = (GB * NIDX + 15) // 16    # wrapped gidx cols per GB group
AGW = (AGB * NAG + 15) // 16    # wrapped agidx cols per AGB group

_CACHE = {}


def _box_geometry(boxes_b):
    fb = boxes_b.astype(np.float32)
    x0 = np.floor(fb[:, 0] * np.float32(W))
    y0 = np.floor(fb[:, 1] * np.float32(H))
    w0 = np.maximum(np.floor(fb[:, 2] * np.float32(W)), np.float32(1.0))
    h0 = np.maximum(np.floor(fb[:, 3] * np.float32(H)), np.float32(1.0))
    grid = (np.arange(S, dtype=np.float32) + np.float32(0.5)) / np.float32(S)
    sy = np.clip(grid[None, :] * h0[:, None] - np.float32(0.5),
                 np.float32(0.0), (h0 - np.float32(1.0))[:, None]) + y0[:, None]
    sx = np.clip(grid[None, :] * w0[:, None] - np.float32(0.5),
                 np.float32(0.0), (w0 - np.float32(1.0))[:, None]) + x0[:, None]
    yf = np.floor(sy)
    xf = np.floor(sx)
    wy = (sy - yf).astype(np.float32)
    wx = (sx - xf).astype(np.float32)
    y0i = np.clip(yf, 0, H - 1).astype(np.int64)
    y1i = np.clip(yf + 1, 0, H - 1).astype(np.int64)
    x0i = np.clip(xf, 0, W - 1).astype(np.int64)
    x1i = np.clip(xf + 1, 0, W - 1).astype(np.int64)
    return wy, wx, y0i, y1i, x0i, x1i


def _wrap16(vals_2d, dtype):
    """[nblk, n] -> [128, nblk*ceil(n/16)]; idx i at [i%16, i//16] per block."""
    nb, n = vals_2d.shape
    sw = (n + 15) // 16
    w = np.zeros((nb, 16, sw), dtype=dtype)
    idx = np.arange(n)
    w[:, idx % 16, idx // 16] = vals_2d
    w = w.transpose(1, 0, 2).reshape(16, nb * sw)
    return np.tile(w, (8, 1))


def _prep_core(image_b, boxes_b):
    """image_b [C,H,W] f32, boxes_b [NBOX,4] f32 -> device input dict."""
    wy, wx, y0i, y1i, x0i, x1i = _box_geometry(boxes_b)

    yb = (y0i.min(axis=1) // STEP) * STEP               # [NBOX], mult of 128
    assert (y1i.max(axis=1) - yb).max() < YW

    # dma_gather idx: n = (b%GB)*NIDX + u*128 + p ; p=j
    xtap = np.stack([x0i, x1i], axis=1)                  # [NBOX, 2(u), S]
    start = xtap * (H * C) + (yb * C)[:, None, None]     # [NBOX, 2, S]
    assert (start % STEP == 0).all()
    gidx = start // STEP
    assert gidx.max() < 32768 and gidx.min() >= 0
    full = np.zeros((NBOX, 2, 128), dtype=np.int16)
    full[:, :, :S] = gidx.astype(np.int16)
    gidx_all = _wrap16(full.reshape(NBOX // GB, GB * NIDX), np.int16)

    # ap_gather idx over hf [AGB, YW, C]: value = (b%AGB)*YW + (ytap-yb)
    ytap = np.stack([y0i, y1i], axis=1)                  # [NBOX, 2(t), S]
    yrel = ytap - yb[:, None, None]                      # [NBOX, 2, S]
    assert yrel.min() >= 0 and yrel.max() < YW
    agv = yrel.reshape(NBOX // AGB, AGB, 2 * S) \
        + (np.arange(AGB) * YW)[None, :, None]
    agidx_all = _wrap16(agv.reshape(NBOX // AGB, AGB * NAG).astype(np.int16),
                        np.int16)

    m1wx = np.zeros((128, NBOX), dtype=np.float32)
    wxT = np.zeros((128, NBOX), dtype=np.float32)
    m1wx[:S] = (np.float32(1.0) - wx).T
    wxT[:S] = wx.T
    wyT = np.zeros((128, NBOX), dtype=np.float32)
    wyT[:S] = wy.T

    img = np.ascontiguousarray(
        image_b.transpose(2, 1, 0)).astype(ml_dtypes.bfloat16)  # [W, H, C]
    img_pad = np.zeros((W * H * C + NPAD,), dtype=ml_dtypes.bfloat16)
    img_pad[:W * H * C] = img.reshape(-1)

    return {
        "img": img_pad.reshape(1, -1),
        "gidx": gidx_all,
        "agidx": agidx_all,
        "m1wx": m1wx,
        "wxT": wxT,
        "wyT": wyT,
    }


def _build_program():
    import concourse.bass as bass
    import concourse.tile as tile
    from concourse import bacc, mybir
    from concourse.masks import make_identity

    bf16 = mybir.dt.bfloat16
    f32 = mybir.dt.float32
    i16 = mybir.dt.int16
    Alu = mybir.AluOpType

    nc = bacc.Bacc("TRN2", target_bir_lowering=False, debug=False,
                   enable_asserts=False, num_devices=8)
    img_d = nc.dram_tensor("img", [1, W * H * C + NPAD], bf16,
                           kind="ExternalInput")
    gidx_d = nc.dram_tensor("gidx", [128, (NBOX // GB) * GIW], i16,
                            kind="ExternalInput")
    agidx_d = nc.dram_tensor("agidx", [128, (NBOX // AGB) * AGW], i16,
                             kind="ExternalInput")
    m1wx_d = nc.dram_tensor("m1wx", [128, NBOX], f32, kind="ExternalInput")
    wxT_d = nc.dram_tensor("wxT", [128, NBOX], f32, kind="ExternalInput")
    wyT_d = nc.dram_tensor("wyT", [128, NBOX], f32, kind="ExternalInput")
    out_d = nc.dram_tensor("out", [NBOX, C, S, S], f32, kind="ExternalOutput")

    with tile.TileContext(nc) as tc, ExitStack() as ctx:
        const = ctx.enter_context(tc.tile_pool(name="const", bufs=1))
        gidx_s = const.tile([128, (NBOX // GB) * GIW], i16)
        nc.sync.dma_start(gidx_s[:], gidx_d.ap())
        agidx_s = const.tile([128, (NBOX // AGB) * AGW], i16)
        nc.sync.dma_start(agidx_s[:], agidx_d.ap())
        m1wx_s = const.tile([128, NBOX], f32)
        nc.sync.dma_start(m1wx_s[:], m1wx_d.ap())
        wxT_s = const.tile([128, NBOX], f32)
        nc.sync.dma_start(wxT_s[:], wxT_d.ap())
        wyT_s = const.tile([128, NBOX], f32)
        nc.sync.dma_start(wyT_s[:], wyT_d.ap())
        ident = const.tile([128, 128], f32)
        make_identity(nc, ident[:])

        nrow = (W * H * C + NPAD - ELEM) // STEP
        in_view = bass.AP(img_d.ap().tensor, 0, [[STEP, nrow], [1, ELEM]])

        gpool = ctx.enter_context(tc.tile_pool(name="g", bufs=2))
        hpool = ctx.enter_context(tc.tile_pool(name="h", bufs=3))
        hfpool = ctx.enter_context(tc.tile_pool(name="hf", bufs=2))
        vpool = ctx.enter_context(tc.tile_pool(name="v", bufs=2))
        opool = ctx.enter_context(tc.tile_pool(name="o", bufs=3))
        otpool = ctx.enter_context(tc.tile_pool(name="ot", bufs=3))
        pspool = ctx.enter_context(tc.tile_pool(name="ps", bufs=2,
                                                space="PSUM"))

        hf_tiles = {}
        G_cur = None
        for m in range(NBOX):
            bg, bo = m // GB, m % GB
            if bo == 0:
                G_cur = gpool.tile([128, GB * 2, ELEM], bf16, tag="G")
                nc.gpsimd.dma_gather(
                    out_ap=G_cur[:], in_ap=in_view,
                    idxs_ap=gidx_s[:, bg * GIW:(bg + 1) * GIW],
                    num_idxs=GB * NIDX, num_idxs_reg=GB * NIDX,
                    elem_size=ELEM, elem_step=STEP,
                )

            # horizontal blend (bf16): h = G_u0*(1-wx) + G_u1*wx
            t = hpool.tile([128, ELEM], bf16, tag="t")
            nc.vector.tensor_scalar(
                out=t[:], in0=G_cur[:, 2 * bo, :],
                scalar1=m1wx_s[:, m:m + 1], scalar2=None, op0=Alu.mult)
            h = hpool.tile([128, ELEM], bf16, tag="h")
            nc.vector.scalar_tensor_tensor(
                out=h[:], in0=G_cur[:, 2 * bo + 1, :],
                scalar=wxT_s[:, m:m + 1], in1=t[:],
                op0=Alu.mult, op1=Alu.add)

            # upcast to f32 on ACT into the AGB-batched table
            ag, ao = m // AGB, m % AGB
            if ao == 0:
                hf_tiles[ag] = hfpool.tile([128, AGB, ELEM], f32, tag="hf", name=f"hf{ag}")
            nc.scalar.copy(out=hf_tiles[ag][:, ao, :], in_=h[:])

            if ao == AGB - 1:
                hf = hf_tiles.pop(ag)
                Hv = vpool.tile([128, AGB, 2, S, C], f32, tag="Hv")
                nc.gpsimd.ap_gather(
                    out_ap=Hv[:].rearrange("p b t i c -> p (b t i) c"),
                    in_ap=hf[:].rearrange("p b e -> p (b e)").rearrange(
                        "p (y c) -> p y c", c=C),
                    idxs_ap=agidx_s[:, ag * AGW:(ag + 1) * AGW],
                    channels=128, num_elems=AGB * YW, d=C,
                    num_idxs=AGB * NAG,
                )
                for m2 in range(ag * AGB, (ag + 1) * AGB):
                    ao2 = m2 % AGB
                    # transpose [j, i] -> [i, j] per (t, c) via PE
                    ps = pspool.tile([S, 2, C, 128], f32, tag="ps")
                    for tt in range(2):
                        for c in range(C):
                            nc.tensor.transpose(
                                out=ps[:, tt, c, :], in_=Hv[:, ao2, tt, :, c],
                                identity=ident[:])
                    HvT = otpool.tile([S, 2, C, S], f32, tag="HvT")
                    nc.scalar.copy(out=HvT[:], in_=ps[:, :, :, :S])

                    # final vertical blend (f32), wy now per-partition (i):
                    # o = HvT0 + (HvT1-HvT0)*wy
                    dv = opool.tile([S, C, S], f32, tag="dv")
                    nc.vector.tensor_tensor(
                        out=dv[:], in0=HvT[:, 1], in1=HvT[:, 0],
                        op=Alu.subtract)
                    o = opool.tile([S, C, S], f32, tag="o")
                    nc.vector.scalar_tensor_tensor(
                        out=o[:], in0=dv[:], scalar=wyT_s[:S, m2:m2 + 1],
                        in1=HvT[:, 0], op0=Alu.mult, op1=Alu.add)

                    dst = out_d.ap()[m2].transpose([1, 0, 2])  # [S(i), C, S(j)]
                    nc.sync.dma_start(dst, o[:])

    nc.compile()
    return nc


def kernel(images: np.ndarray, boxes: np.ndarray) -> np.ndarray:
    images = np.asarray(images, dtype=np.float32)
    boxes = np.asarray(boxes, dtype=np.float32)
    assert images.shape == (B, C, H, W) and boxes.shape == (B, NBOX, 4)

    if "nc" not in _CACHE:
        _CACHE["nc"] = _build_program()
    nc = _CACHE["nc"]

    in_maps = [_prep_core(images[b], boxes[b]) for b in range(B)]

    from concourse.bass_utils import run_bass_kernel_spmd
    res = run_bass_kernel_spmd(nc, in_maps, core_ids=list(range(B)))
    out = np.stack([res.results[b]["out"] for b in range(B)], axis=0)
    return out.reshape(B * NBOX, C, S, S)


if __name__ == "__main__":
    import reference
    inputs = {k: np.asarray(v) for k, v in reference.setup_inputs().items()}
    got = kernel(**inputs)
    expected = np.asarray(reference.reference(**inputs))
    err = np.abs(got - expected)
    denom = np.abs(expected).max()
    print("max abs err:", err.max(), " rel:", err.max() / denom)
